# revision 14
# baseline (speedup 1.0000x reference)
"""ActorCriticLoss (TD-lambda + symlog critic) on 8 Trainium2 NeuronCores.

Data-parallel over the batch axis (65536 -> 8 x 8192). The device reduces
each shard to per-partition partials; the O(1) loss assembly runs on the
host in float64.

Math: with phi_t = ret_t + (K1/K2) v_t the TD(lambda) recurrence becomes
  phi_t = a_t + K2 c_t phi_{t+1},   a_t = r_t + (K1/K2) v_t
(the c*v_next product cancels). The backward scan runs as one forward
`tensor_tensor_scan` per tile over per-row padded, time-reversed streams
([pad, t=63..0] per row, k_pad = 0, a_pad = bootstrap*(1+K1/K2)) so the
fp32 scan carry reinitializes at every row boundary. The device works with
retm = -ret throughout; signs are fixed on the host.

Split of labor (driven by measured per-instruction HW costs):
 - HOST (numpy, exact fp32/f64): builds the padded bf16 a/k streams and
   time-reversed bf16 v/log_probs; computes sum(entropy) and sum(lp*v)
   from the fp32 originals (entropy/rewards/continues never hit the device
   elementwise paths they aren't needed in).
 - DVE (all-bf16, 2x rate; fp32 scan carry + fp32 accumulators): scan,
   retm, sum(lp*retm), sign-copies via uint16 bit ops, d, min/max.
 - ACT: |v|, ln(1+|v|), |ret|, ln(1+|ret|), v->bf16, sum(d^2) via
   Square+accum (bf16 in, fp32 accumulate).
 - PE: sum(lp) via ones-matmul into PSUM.
"""

import sys

import ml_dtypes
import numpy as np

sys.path.insert(0, "/opt/trn_rl_repo")

import concourse.bass as bass  # noqa: E402
import concourse.mybir as mybir  # noqa: E402
import concourse.tile as tile  # noqa: E402
from concourse import bacc  # noqa: E402
from concourse.bass_utils import run_bass_kernel_spmd  # noqa: E402

B, T = 65536, 64
NCORES = 8
B_LOC = B // NCORES
P = 128
M = 16                       # rows per partition per tile
NT = B_LOC // (P * M)
F = M * T                    # payload elements/partition per tile
S = T + 1                    # padded slots per row
FP = M * S

DISCOUNT, LAMBDA = 0.997, 0.95
ENTROPY_SCALE = 0.0003
RETURN_EMA_DECAY = 0.99
K2 = DISCOUNT * LAMBDA
RATIO = (1.0 - LAMBDA) / LAMBDA

f32 = mybir.dt.float32
bf16 = mybir.dt.bfloat16
u16 = mybir.dt.uint16
AX = mybir.AxisListType
OP = mybir.AluOpType
AF = mybir.ActivationFunctionType
BF = ml_dtypes.bfloat16

# acc_dve columns: mxm(0:NT) mnm(NT:2NT); acc_act: d2(0:NT)
N_DVE = 2 * NT
N_ACT = NT
N_OUT = N_DVE + N_ACT
PE_N = 512


def _stt_uint_imm(eng, out, in0, imm, in1, op0, op1, imm_dtype=u16,
                  accum_out=None):
    """scalar_tensor_tensor with an integer-typed immediate (the public
    wrapper emits float32 immediates; bitvec ops need the immediate typed
    like src/dst)."""
    outs = [eng.lower_ap(out)]
    if accum_out is not None:
        outs.append(eng.lower_ap(accum_out))
    return eng.add_instruction(
        mybir.InstTensorScalarPtr(
            name=eng.bass.get_next_instruction_name(),
            is_scalar_tensor_tensor=True,
            op0=op0,
            op1=op1,
            ins=[
                eng.lower_ap(in0),
                mybir.ImmediateValue(dtype=imm_dtype, value=imm),
                eng.lower_ap(in1),
            ],
            outs=outs,
        )
    )


def build_module():
    nc = bacc.Bacc(
        "TRN2", target_bir_lowering=False, debug=False, enable_asserts=False
    )
    a_d = nc.dram_tensor("a_pad", [B_LOC, S], bf16, kind="ExternalInput").ap()
    k_d = nc.dram_tensor("k_pad", [B_LOC, S], bf16, kind="ExternalInput").ap()
    v_d = nc.dram_tensor("vs_rev", [B_LOC, T], bf16, kind="ExternalInput").ap()
    lp_d = nc.dram_tensor("log_probs_rev", [B_LOC, T], bf16,
                          kind="ExternalInput").ap()
    out_d = nc.dram_tensor("out", [P, N_OUT], f32, kind="ExternalOutput").ap()
    pe_d = nc.dram_tensor("pe_out", [1, 2 * PE_N], f32, kind="ExternalOutput").ap()

    a_v = a_d.rearrange("(n p m) s -> n p (m s)", p=P, m=M)
    k_v = k_d.rearrange("(n p m) s -> n p (m s)", p=P, m=M)
    v_v = v_d.rearrange("(n p m) t -> n p (m t)", p=P, m=M)
    lp_v = lp_d.rearrange("(n p m) t -> n p (m t)", p=P, m=M)

    with tile.TileContext(nc) as tc:
        with (
            tc.tile_pool(name="const", bufs=1) as constp,
            tc.tile_pool(name="ins", bufs=3) as ins,
            tc.tile_pool(name="ins2", bufs=3) as ins2,
            tc.tile_pool(name="work", bufs=3) as work,
            tc.tile_pool(name="accp", bufs=1) as accp,
            tc.tile_pool(name="psum", bufs=1, space="PSUM") as psp,
        ):
            acc_dve = accp.tile([P, N_DVE], f32)
            acc_act = accp.tile([P, N_ACT], f32)
            ones = constp.tile([P, 1], bf16)
            nc.gpsimd.memset(ones[:], 1.0)
            ps_lp = psp.tile([1, PE_N], f32)
            ps_u1 = psp.tile([1, PE_N], f32)

            for n in range(NT):
                a_t = ins.tile([P, FP], bf16)
                k_t = ins.tile([P, FP], bf16)
                v_t = ins2.tile([P, F], bf16)
                lp_t = ins2.tile([P, F], bf16)
                nc.sync.dma_start(a_t[:], a_v[n])
                nc.sync.dma_start(k_t[:], k_v[n])
                nc.gpsimd.dma_start(v_t[:], v_v[n])
                nc.gpsimd.dma_start(lp_t[:], lp_v[n])

                phi_t = work.tile([P, FP], bf16)
                retm_t = work.tile([P, F], bf16)
                av_t = work.tile([P, F], bf16)
                lnv_t = work.tile([P, F], bf16)
                ar_t = work.tile([P, F], bf16)
                lnr_t = work.tile([P, F], bf16)
                d_t = work.tile([P, F], bf16)
                j1_t = work.tile([P, F], bf16)

                v3 = v_t[:].rearrange("p (m t) -> p m t", t=T)
                phi3 = phi_t[:].rearrange("p (m s) -> p m s", s=S)
                retm3 = retm_t[:].rearrange("p (m t) -> p m t", t=T)
                phi_pay = phi3[:, :, 1:S]  # payload slots, stream order

                # DVE: TD(lambda) scan, one instruction per tile
                nc.vector.tensor_tensor_scan(
                    phi_t[:], k_t[:], a_t[:], 0.0, OP.mult, OP.add
                )
                # DVE: retm = vs - phi (= -ret); vs = RATIO*v from host
                nc.vector.tensor_sub(retm3, v3, phi_pay)
                # DVE: extrema of retm via one bf16 tree level + reduce
                mx1_t = work.tile([P, F // 2], bf16)
                mn1_t = work.tile([P, F // 2], bf16)
                half = F // 2
                nc.vector.tensor_tensor(
                    mx1_t[:], retm_t[:, 0:half], retm_t[:, half:F], op=OP.max
                )
                nc.vector.tensor_tensor(
                    mn1_t[:], retm_t[:, 0:half], retm_t[:, half:F], op=OP.min
                )
                nc.vector.tensor_reduce(
                    acc_dve[:, n : n + 1], mx1_t[:], axis=AX.X, op=OP.max,
                )
                nc.vector.tensor_reduce(
                    acc_dve[:, NT + n : NT + n + 1], mn1_t[:],
                    axis=AX.X, op=OP.min,
                )
                # DVE: lp*retm product; PE: its sum + sum(lp)
                nc.vector.tensor_mul(j1_t[:], lp_t[:], retm_t[:])
                for h in range(F // PE_N):
                    first = n == 0 and h == 0
                    last = n == NT - 1 and h == F // PE_N - 1
                    sl = slice(h * PE_N, (h + 1) * PE_N)
                    nc.tensor.matmul(
                        ps_lp[:], ones[:], lp_t[:, sl], start=first, stop=last
                    )
                    nc.tensor.matmul(
                        ps_u1[:], ones[:], j1_t[:, sl], start=first, stop=last
                    )
                # ACT: symlog magnitudes (bf16 in/out)
                nc.scalar.activation(av_t[:], v_t[:], AF.Abs, scale=1.0 / RATIO)
                nc.scalar.activation(lnv_t[:], av_t[:], AF.Ln, bias=1.0)
                nc.scalar.activation(ar_t[:], retm_t[:], AF.Abs)
                nc.scalar.activation(lnr_t[:], ar_t[:], AF.Ln, bias=1.0)
                # DVE: signed symlogs via uint16 sign-bit copy
                _stt_uint_imm(
                    nc.vector, av_t[:].bitcast(u16), v_t[:].bitcast(u16),
                    0x8000, lnv_t[:].bitcast(u16),
                    OP.bitwise_and, OP.bitwise_or,
                )
                _stt_uint_imm(
                    nc.vector, ar_t[:].bitcast(u16), retm_t[:].bitcast(u16),
                    0x8000, lnr_t[:].bitcast(u16),
                    OP.bitwise_and, OP.bitwise_or,
                )
                # DVE: d = sv + sr' (= symlog v - symlog ret)
                nc.vector.tensor_add(d_t[:], av_t[:], ar_t[:])
                # ACT: critic partial sums (fp32 accumulate)
                nc.scalar.activation(
                    j1_t[:], d_t[:], AF.Square,
                    accum_out=acc_act[:, n : n + 1],
                )

            pe_sb = accp.tile([1, 2 * PE_N], f32)
            nc.scalar.copy(pe_sb[:, 0:PE_N], ps_lp[:])
            nc.scalar.copy(pe_sb[:, PE_N:], ps_u1[:])
            nc.sync.dma_start(out_d[:, 0:N_DVE], acc_dve[:])
            nc.sync.dma_start(out_d[:, N_DVE:N_OUT], acc_act[:])
            nc.sync.dma_start(pe_d, pe_sb[:])

    nc.compile()
    return nc


_NC = None


def _get_nc():
    global _NC
    if _NC is None:
        _NC = build_module()
    return _NC


def _run(in_maps, trace=False, **kwargs):
    return run_bass_kernel_spmd(
        _get_nc(), in_maps, core_ids=list(range(NCORES)), trace=trace, **kwargs
    )


def prepare(rewards, values, continues, bootstrap, log_probs, entropy):
    """Host prep: padded reversed bf16 scan streams + reversed bf16 v/lp,
    plus the exact host-side sums that never need the device."""
    r = np.asarray(rewards, dtype=np.float32)
    v = np.asarray(values, dtype=np.float32)
    c = np.asarray(continues, dtype=np.float32)
    bs = np.asarray(bootstrap, dtype=np.float32)
    lp = np.asarray(log_probs, dtype=np.float32)
    en = np.asarray(entropy, dtype=np.float32)

    a_pad = np.empty((B, S), dtype=BF)
    a_pad[:, 0] = (bs * np.float32(1.0 + RATIO)).astype(BF)
    a_pad[:, 1:] = (r + np.float32(RATIO) * v)[:, ::-1].astype(BF)
    k_pad = np.empty((B, S), dtype=BF)
    k_pad[:, 0] = BF(0.0)
    k_pad[:, 1:] = (np.float32(K2) * c)[:, ::-1].astype(BF)
    vs_rev = np.ascontiguousarray((np.float32(RATIO) * v)[:, ::-1]).astype(BF)
    lp_rev = np.ascontiguousarray(lp[:, ::-1]).astype(BF)

    host = {
        "u2": np.dot(
            lp.ravel().astype(np.float64), v.ravel().astype(np.float64)
        ),
        "sent": en.sum(dtype=np.float64),
    }

    in_maps = []
    for i in range(NCORES):
        sl = slice(i * B_LOC, (i + 1) * B_LOC)
        in_maps.append(
            {
                "a_pad": np.ascontiguousarray(a_pad[sl]),
                "k_pad": np.ascontiguousarray(k_pad[sl]),
                "vs_rev": np.ascontiguousarray(vs_rev[sl]),
                "log_probs_rev": np.ascontiguousarray(lp_rev[sl]),
            }
        )
    return in_maps, host


def combine(results, host):
    outs = np.stack([res["out"] for res in results]).astype(np.float64)
    pe = np.stack([res["pe_out"] for res in results]).astype(np.float64)
    mn = -outs[:, :, 0:NT].max()             # min ret
    mx = -outs[:, :, NT : 2 * NT].min()      # max ret
    d2 = outs[:, :, N_DVE:].sum()
    slp = pe[:, 0, 0:PE_N].sum()
    u1 = -pe[:, 0, PE_N:].sum()              # sum lp*ret
    u2 = host["u2"]
    sent = host["sent"]

    n = float(B * T)
    ema = 1.0 - RETURN_EMA_DECAY
    lo_n = ema * mn
    hi_n = 1.0 + ema * (mx - 1.0)
    scale = max(hi_n - lo_n, 1.0)
    pg = -((u1 / n) / scale - lo_n * (slp / n) / scale - (u2 / n))
    entropy_loss = -ENTROPY_SCALE * (sent / n)
    critic = d2 / n
    return np.float32(pg + entropy_loss + critic)


def kernel(rewards, values, continues, bootstrap, log_probs, entropy):
    in_maps, host = prepare(
        rewards, values, continues, bootstrap, log_probs, entropy
    )
    results = _run(in_maps).results
    return combine(results, host)


# revision 15
# speedup vs baseline: 1.0405x; 1.0405x over previous
"""ActorCriticLoss (TD-lambda + symlog critic) on 8 Trainium2 NeuronCores.

Data-parallel over the batch axis (65536 -> 8 x 8192). The device reduces
each shard to per-partition partials; the O(1) loss assembly runs on the
host in float64.

Math: with phi_t = ret_t + (K1/K2) v_t the TD(lambda) recurrence becomes
  phi_t = a_t + K2 c_t phi_{t+1},   a_t = r_t + (K1/K2) v_t
(the c*v_next product cancels). The backward scan runs as one forward
`tensor_tensor_scan` per tile over per-row padded, time-reversed streams
([pad, t=63..0] per row, k_pad = 0, a_pad = bootstrap*(1+K1/K2)) so the
fp32 scan carry reinitializes at every row boundary. The device works with
retm = -ret throughout; signs are fixed on the host.

Split of labor (driven by measured per-instruction HW costs):
 - HOST (numpy, exact fp32/f64): builds the padded bf16 a/k streams and
   time-reversed bf16 v/log_probs; computes sum(entropy) and sum(lp*v)
   from the fp32 originals (entropy/rewards/continues never hit the device
   elementwise paths they aren't needed in).
 - DVE (all-bf16, 2x rate; fp32 scan carry + fp32 accumulators): scan,
   retm, sum(lp*retm), sign-copies via uint16 bit ops, d, min/max.
 - ACT: |v|, ln(1+|v|), |ret|, ln(1+|ret|), v->bf16, sum(d^2) via
   Square+accum (bf16 in, fp32 accumulate).
 - PE: sum(lp) via ones-matmul into PSUM.
"""

import sys

import ml_dtypes
import numpy as np

sys.path.insert(0, "/opt/trn_rl_repo")

import concourse.bass as bass  # noqa: E402
import concourse.mybir as mybir  # noqa: E402
import concourse.tile as tile  # noqa: E402
from concourse import bacc  # noqa: E402
from concourse.bass_utils import run_bass_kernel_spmd  # noqa: E402

B, T = 65536, 64
NCORES = 8
B_LOC = B // NCORES
P = 128
M_LIST = [8, 16, 16, 16, 8]  # rows/partition per tile (small first+last
                             # tiles: faster pipeline start and drain)
NT = len(M_LIST)
assert sum(M_LIST) * P == B_LOC
S = T + 1                    # padded slots per row

DISCOUNT, LAMBDA = 0.997, 0.95
ENTROPY_SCALE = 0.0003
RETURN_EMA_DECAY = 0.99
K2 = DISCOUNT * LAMBDA
RATIO = (1.0 - LAMBDA) / LAMBDA

f32 = mybir.dt.float32
bf16 = mybir.dt.bfloat16
u16 = mybir.dt.uint16
AX = mybir.AxisListType
OP = mybir.AluOpType
AF = mybir.ActivationFunctionType
BF = ml_dtypes.bfloat16

# acc_dve columns: mxm(0:NT) mnm(NT:2NT); acc_act: d2(0:NT)
N_DVE = 2 * NT
N_ACT = NT
N_OUT = N_DVE + N_ACT
PE_N = 512


def _stt_uint_imm(eng, out, in0, imm, in1, op0, op1, imm_dtype=u16,
                  accum_out=None):
    """scalar_tensor_tensor with an integer-typed immediate (the public
    wrapper emits float32 immediates; bitvec ops need the immediate typed
    like src/dst)."""
    outs = [eng.lower_ap(out)]
    if accum_out is not None:
        outs.append(eng.lower_ap(accum_out))
    return eng.add_instruction(
        mybir.InstTensorScalarPtr(
            name=eng.bass.get_next_instruction_name(),
            is_scalar_tensor_tensor=True,
            op0=op0,
            op1=op1,
            ins=[
                eng.lower_ap(in0),
                mybir.ImmediateValue(dtype=imm_dtype, value=imm),
                eng.lower_ap(in1),
            ],
            outs=outs,
        )
    )


def build_module():
    nc = bacc.Bacc(
        "TRN2", target_bir_lowering=False, debug=False, enable_asserts=False
    )
    a_d = nc.dram_tensor("a_pad", [B_LOC, S], bf16, kind="ExternalInput").ap()
    k_d = nc.dram_tensor("k_pad", [B_LOC, S], bf16, kind="ExternalInput").ap()
    v_d = nc.dram_tensor("vs_rev", [B_LOC, T], bf16, kind="ExternalInput").ap()
    lp_d = nc.dram_tensor("log_probs_rev", [B_LOC, T], bf16,
                          kind="ExternalInput").ap()
    out_d = nc.dram_tensor("out", [P, N_OUT], f32, kind="ExternalOutput").ap()
    pe_d = nc.dram_tensor("pe_out", [1, 2 * PE_N], f32, kind="ExternalOutput").ap()



    with tile.TileContext(nc) as tc:
        with (
            tc.tile_pool(name="const", bufs=1) as constp,
            tc.tile_pool(name="ins", bufs=3) as ins,
            tc.tile_pool(name="ins2", bufs=3) as ins2,
            tc.tile_pool(name="work", bufs=3) as work,
            tc.tile_pool(name="accp", bufs=1) as accp,
            tc.tile_pool(name="psum", bufs=1, space="PSUM") as psp,
        ):
            acc_dve = accp.tile([P, N_DVE], f32)
            acc_act = accp.tile([P, N_ACT], f32)
            ones = constp.tile([P, 1], bf16)
            nc.gpsimd.memset(ones[:], 1.0)
            ps_lp = psp.tile([1, PE_N], f32)
            ps_u1 = psp.tile([1, PE_N], f32)

            row0 = 0
            for n, Mn in enumerate(M_LIST):
                F = Mn * T
                FP = Mn * S
                rows = slice(row0 * P, (row0 + Mn) * P)
                row0 += Mn
                a_vn = a_d[rows].rearrange("(p m) s -> p (m s)", p=P)
                k_vn = k_d[rows].rearrange("(p m) s -> p (m s)", p=P)
                v_vn = v_d[rows].rearrange("(p m) t -> p (m t)", p=P)
                lp_vn = lp_d[rows].rearrange("(p m) t -> p (m t)", p=P)

                a_t = ins.tile([P, FP], bf16, tag="a", name=f"a{n}")
                k_t = ins.tile([P, FP], bf16, tag="k", name=f"k{n}")
                v_t = ins2.tile([P, F], bf16, tag="v", name=f"v{n}")
                lp_t = ins2.tile([P, F], bf16, tag="lp", name=f"lp{n}")
                nc.sync.dma_start(a_t[:], a_vn)
                nc.sync.dma_start(k_t[:], k_vn)
                nc.sync.dma_start(v_t[:], v_vn)
                nc.sync.dma_start(lp_t[:], lp_vn)

                phi_t = work.tile([P, FP], bf16, tag="phi", name=f"phi{n}")
                retm_t = work.tile([P, F], bf16, tag="retm", name=f"retm{n}")
                av_t = work.tile([P, F], bf16, tag="av", name=f"av{n}")
                lnv_t = work.tile([P, F], bf16, tag="lnv", name=f"lnv{n}")
                ar_t = work.tile([P, F], bf16, tag="ar", name=f"ar{n}")
                lnr_t = work.tile([P, F], bf16, tag="lnr", name=f"lnr{n}")
                d_t = work.tile([P, F], bf16, tag="d", name=f"d{n}")
                j1_t = work.tile([P, F], bf16, tag="j1", name=f"j1{n}")

                v3 = v_t[:].rearrange("p (m t) -> p m t", t=T)
                phi3 = phi_t[:].rearrange("p (m s) -> p m s", s=S)
                retm3 = retm_t[:].rearrange("p (m t) -> p m t", t=T)
                phi_pay = phi3[:, :, 1:S]  # payload slots, stream order

                # DVE: TD(lambda) scan, one instruction per tile
                nc.vector.tensor_tensor_scan(
                    phi_t[:], k_t[:], a_t[:], 0.0, OP.mult, OP.add
                )
                # DVE: retm = vs - phi (= -ret); vs = RATIO*v from host
                nc.vector.tensor_sub(retm3, v3, phi_pay)
                # DVE: extrema of retm via one bf16 tree level + reduce
                mx1_t = work.tile([P, F // 2], bf16, tag="mx1", name=f"mx1{n}")
                mn1_t = work.tile([P, F // 2], bf16, tag="mn1", name=f"mn1{n}")
                half = F // 2
                nc.vector.tensor_tensor(
                    mx1_t[:], retm_t[:, 0:half], retm_t[:, half:F], op=OP.max
                )
                nc.vector.tensor_tensor(
                    mn1_t[:], retm_t[:, 0:half], retm_t[:, half:F], op=OP.min
                )
                nc.vector.tensor_reduce(
                    acc_dve[:, n : n + 1], mx1_t[:], axis=AX.X, op=OP.max,
                )
                nc.vector.tensor_reduce(
                    acc_dve[:, NT + n : NT + n + 1], mn1_t[:],
                    axis=AX.X, op=OP.min,
                )
                # DVE: lp*retm product; PE: its sum + sum(lp)
                nc.vector.tensor_mul(j1_t[:], lp_t[:], retm_t[:])
                nch = max(F // PE_N, 1)
                for h in range(nch):
                    first = n == 0 and h == 0
                    last = n == NT - 1 and h == nch - 1
                    sl = slice(h * PE_N, min((h + 1) * PE_N, F))
                    nc.tensor.matmul(
                        ps_lp[:, 0 : sl.stop - sl.start], ones[:],
                        lp_t[:, sl], start=first, stop=last
                    )
                    nc.tensor.matmul(
                        ps_u1[:, 0 : sl.stop - sl.start], ones[:],
                        j1_t[:, sl], start=first, stop=last
                    )
                # ACT: symlog magnitudes (bf16 in/out)
                nc.scalar.activation(av_t[:], v_t[:], AF.Abs, scale=1.0 / RATIO)
                nc.scalar.activation(lnv_t[:], av_t[:], AF.Ln, bias=1.0)
                nc.scalar.activation(ar_t[:], retm_t[:], AF.Abs)
                nc.scalar.activation(lnr_t[:], ar_t[:], AF.Ln, bias=1.0)
                # DVE: signed symlogs via uint16 sign-bit copy
                _stt_uint_imm(
                    nc.vector, av_t[:].bitcast(u16), v_t[:].bitcast(u16),
                    0x8000, lnv_t[:].bitcast(u16),
                    OP.bitwise_and, OP.bitwise_or,
                )
                _stt_uint_imm(
                    nc.vector, ar_t[:].bitcast(u16), retm_t[:].bitcast(u16),
                    0x8000, lnr_t[:].bitcast(u16),
                    OP.bitwise_and, OP.bitwise_or,
                )
                # DVE: d = sv + sr' (= symlog v - symlog ret)
                nc.vector.tensor_add(d_t[:], av_t[:], ar_t[:])
                # ACT: critic partial sums (fp32 accumulate)
                nc.scalar.activation(
                    j1_t[:], d_t[:], AF.Square,
                    accum_out=acc_act[:, n : n + 1],
                )

            pe_sb = accp.tile([1, 2 * PE_N], f32)
            nc.scalar.copy(pe_sb[:, 0:PE_N], ps_lp[:])
            nc.scalar.copy(pe_sb[:, PE_N:], ps_u1[:])
            nc.sync.dma_start(out_d[:, 0:N_DVE], acc_dve[:])
            nc.sync.dma_start(out_d[:, N_DVE:N_OUT], acc_act[:])
            nc.sync.dma_start(pe_d, pe_sb[:])

    nc.compile()
    return nc


_NC = None


def _get_nc():
    global _NC
    if _NC is None:
        _NC = build_module()
    return _NC


def _run(in_maps, trace=False, **kwargs):
    return run_bass_kernel_spmd(
        _get_nc(), in_maps, core_ids=list(range(NCORES)), trace=trace, **kwargs
    )


def prepare(rewards, values, continues, bootstrap, log_probs, entropy):
    """Host prep: padded reversed bf16 scan streams + reversed bf16 v/lp,
    plus the exact host-side sums that never need the device."""
    r = np.asarray(rewards, dtype=np.float32)
    v = np.asarray(values, dtype=np.float32)
    c = np.asarray(continues, dtype=np.float32)
    bs = np.asarray(bootstrap, dtype=np.float32)
    lp = np.asarray(log_probs, dtype=np.float32)
    en = np.asarray(entropy, dtype=np.float32)

    a_pad = np.empty((B, S), dtype=BF)
    a_pad[:, 0] = (bs * np.float32(1.0 + RATIO)).astype(BF)
    a_pad[:, 1:] = (r + np.float32(RATIO) * v)[:, ::-1].astype(BF)
    k_pad = np.empty((B, S), dtype=BF)
    k_pad[:, 0] = BF(0.0)
    k_pad[:, 1:] = (np.float32(K2) * c)[:, ::-1].astype(BF)
    vs_rev = np.ascontiguousarray((np.float32(RATIO) * v)[:, ::-1]).astype(BF)
    lp_rev = np.ascontiguousarray(lp[:, ::-1]).astype(BF)

    host = {
        "u2": np.dot(
            lp.ravel().astype(np.float64), v.ravel().astype(np.float64)
        ),
        "sent": en.sum(dtype=np.float64),
    }

    in_maps = []
    for i in range(NCORES):
        sl = slice(i * B_LOC, (i + 1) * B_LOC)
        in_maps.append(
            {
                "a_pad": np.ascontiguousarray(a_pad[sl]),
                "k_pad": np.ascontiguousarray(k_pad[sl]),
                "vs_rev": np.ascontiguousarray(vs_rev[sl]),
                "log_probs_rev": np.ascontiguousarray(lp_rev[sl]),
            }
        )
    return in_maps, host


def combine(results, host):
    outs = np.stack([res["out"] for res in results]).astype(np.float64)
    pe = np.stack([res["pe_out"] for res in results]).astype(np.float64)
    mn = -outs[:, :, 0:NT].max()             # min ret
    mx = -outs[:, :, NT : 2 * NT].min()      # max ret
    d2 = outs[:, :, N_DVE:].sum()
    slp = pe[:, 0, 0:PE_N].sum()
    u1 = -pe[:, 0, PE_N:].sum()              # sum lp*ret
    u2 = host["u2"]
    sent = host["sent"]

    n = float(B * T)
    ema = 1.0 - RETURN_EMA_DECAY
    lo_n = ema * mn
    hi_n = 1.0 + ema * (mx - 1.0)
    scale = max(hi_n - lo_n, 1.0)
    pg = -((u1 / n) / scale - lo_n * (slp / n) / scale - (u2 / n))
    entropy_loss = -ENTROPY_SCALE * (sent / n)
    critic = d2 / n
    return np.float32(pg + entropy_loss + critic)


def kernel(rewards, values, continues, bootstrap, log_probs, entropy):
    in_maps, host = prepare(
        rewards, values, continues, bootstrap, log_probs, entropy
    )
    results = _run(in_maps).results
    return combine(results, host)
